# revision 20
# baseline (speedup 1.0000x reference)
"""Trainium2 Bass kernel for nn_ModalDecoder (embedding_lookup).

Reference computation:
    w  = out_projection_table[idx].reshape(B, F, D, O)      # [B,F,D,O]
    b  = feature_bias_table[idx]                            # [B,F,D]
    xb = x[:, :, None, :] + b[:, None, :, :]                # [B,N,F,D]
    out = einsum('bnfd,bfdo->bnfo', xb, w)                  # [B,N,F,O]

Factorization (avoids the 128MB [B,N,F,D] intermediate):
    out[b, n, f, :] = x[b, n, :] @ W[b, f] + (bias[b, f] @ W[b, f])
The bias term is a per-(b,f) length-O vector, broadcast over n; it is
precomputed on host (tiny) and added on-device per PSUM tile via
tensor_scalar_add (per-partition scalar), which doubles as the mandatory
PSUM->SBUF copy (DMA cannot read PSUM).

Sharding: 8 cores = 4 values of b x 2 halves of N. Per core:
    y[fo, n] = Wpack[d, fo].T @ xT[d, n] + cvec[fo]
with Wpack = [D, F*O] (host-gathered tables packed side by side), xT the
transposed x half, both bf16 (PSUM accumulates fp32). y is [F*O, NH] bf16
(host upcasts; bf16 output keeps rel err ~3e-3 vs the 2e-2 gate and halves
store traffic to 1MB/core).

Schedule (from perfetto analysis; ~7us of framework preamble and ~1.7us of
epilogue bracket everything):
  * ALL bulk load data lives in ONE packed DRAM tensor on the sync HWDGE
    ring. Descriptor generation costs ~0.65us per dma_start and the
    generation backend is shared between the two HWDGE rings (splitting
    loads across rings parallelizes nothing - measured repeatedly), so
    chunk count and boundaries matter, not tensor boundaries. The first
    3072 columns are four per-k regions [wp_s0_k | xt_k_n0 | xt_k_n1 |
    wp_s1_k] so that chunk A (one 4x768B-run strided DMA, 384KB) delivers
    wp0 + the xt n0-half and gates group 0, chunk B (same shape) delivers
    the xt n1-half + wp1 and gates both s0u1 and s1, while the N=512 rhs
    (both xt halves) stays contiguous per k. wp for s2..s7 follows in
    three 256KB chunks that pace groups 2-7 just ahead of the PE. 5 gens
    instead of 7 pulls every gate ~0.6us earlier.
  * cv rides the scalar ring alone (it gates every DVE add; behind the wp
    bytes it arrives microseconds late and stalls the whole add pipeline).
  * Group 0 runs as two N=256 half-groups (start on 384KB landed);
    groups 1-6 run as N=512 (warm pace 216ns vs 2x131ns - LDWEIGHTS is
    not fully hidden at N=256).
  * Group 7 runs as two N=256 half-groups with the n0 half accumulated in
    bank 0 (closed and drained ~5us earlier; a PE wait on s_dve_act guards
    the reuse), so its DVE add + store issue overlap the last 4 matmuls
    and only the 64KB n1 half sits on the post-matmul tail.
  * The PE is pre-warmed with N=128 dummy matmuls (cold issue ~107ns each)
    sized to run FLUSH into the first gate (~4.1us), so the HAM clock gate
    flips to 2.4GHz before the first real group and every real matmul runs
    warm. Even a ~0.5us warmup->gate idle gap can leave HAM cold for
    microseconds (free-running window phase); undersizing by 2us once
    cost ~4us of half-rate matmuls.
  * DVE does one 512-wide add per group (751ns+overhead < 864ns PE group
    pace; two 256-wide halves per group measured ~1.0us/group and built a
    ~1.8us tail backlog). Group 7 keeps 256-wide half-adds for the split
    tail.
  * Stores alternate rings: scalar takes s0-s3 + s7n0, sync takes s4-s6 +
    s7n1 after its loads.
  * No end-of-kernel waits or sem clears: the NEFF epilogue drains DMAs
    and re-zeroes every semaphore, overlapping the last store transfer.

Per-core HBM traffic: 1.5MB packed loads + 1MB out (memory-bound).
"""

import numpy as np
import ml_dtypes

B, N, D, O, F, V = 4, 1024, 512, 64, 16, 64
NH = N // 2            # 512 rows of x per core
FO = F * O             # 1024 packed output columns
KT = D // 128          # 4 contraction chunks
ST = FO // 128         # 8 output-partition chunks
NQ = NH // 2           # 256 (n-half within a core, group-0/7 split)
N_WARM = 40            # short N=128 PE warmup matmuls during load phase

_cache: dict = {}


def _build_program(with_clears=True):
    # with_clears=True is the real (HW) program. The False variant is for
    # CoreSim validation: it enables the race detector and memsets the
    # warmup scratch (CoreSim rejects reads of uninitialized SBUF; on HW
    # the warmup matmul inputs are garbage by design and never observed).
    import concourse.bass as bass
    import concourse.mybir as mybir

    bf16 = mybir.dt.bfloat16
    f32 = mybir.dt.float32

    nc = bass.Bass(
        "TRN2",
        target_bir_lowering=False,
        debug=False,
        num_devices=8,
        detect_race_conditions=not with_clears,
    )

    # Packed load tensor, 6144 columns. First 3072: four per-k regions of
    # 768 cols [wp_s0_k (128) | xt_k_n0 (256) | xt_k_n1 (256) | wp_s1_k
    # (128)], so ONE 4-run strided DMA covers wp0+xt_n0 (chunk A, gates
    # group 0) and another covers xt_n1+wp1 (chunk B, gates s0u1 AND s1),
    # while the N=512 rhs (xt both halves) stays contiguous per k.
    # Last 3072: wp for s=2..7 as s*512+k*128+c.
    ld_d = nc.dram_tensor("ld", [128, 16, 384], bf16, kind="ExternalInput")
    cv_d = nc.dram_tensor("cv", [128, ST], f32, kind="ExternalInput")
    y_d = nc.dram_tensor("y", [FO, NH], bf16, kind="ExternalOutput")

    yv = y_d.ap().rearrange("(g p) n -> p g n", p=128)  # [128, ST, NH]

    with (
        nc.sbuf_tensor("ld_sb", [128, 16, 384], bf16) as ld_sb,
        nc.sbuf_tensor("cv_sb", [128, ST], f32) as cv_sb,
        nc.sbuf_tensor("out_sb", [128, ST, NH], bf16) as out_sb,
        nc.sbuf_tensor("scr_sb", [128, 128], bf16) as scr_sb,
        nc.psum_tensor([128, ST, NH], f32) as ps,
        nc.semaphore("s_A") as s_A,
        nc.semaphore("s_B") as s_B,
        nc.semaphore("s_wp23") as s_wp23,
        nc.semaphore("s_wp45") as s_wp45,
        nc.semaphore("s_wp67") as s_wp67,
        nc.semaphore("s_cv") as s_cv,
        nc.semaphore("s_mm") as s_mm,
        nc.semaphore("s_dve_sync") as s_dve_sync,
        nc.semaphore("s_dve_act") as s_dve_act,
        nc.semaphore("s_st_sync") as s_st_sync,   # store completion (never waited)
        nc.semaphore("s_st_act") as s_st_act,
        nc.semaphore("s_ws") as s_ws,
        nc.Block() as block,
    ):

        @block.sync
        def _(sync):
            lv = ld_sb.ap().rearrange("p (g h) c -> p g h c", h=2)
            dv = ld_d.ap().rearrange("p (g h) c -> p g h c", h=2)
            # chunk A: [wp_s0_k | xt_k_n0] for all k (4x768B runs, 384KB)
            sync.dma_start(lv[:, 0:4, 0, :], dv[:, 0:4, 0, :]).then_inc(s_A, 16)
            # chunk B: [xt_k_n1 | wp_s1_k] for all k (gates s0u1 AND s1)
            sync.dma_start(lv[:, 0:4, 1, :], dv[:, 0:4, 1, :]).then_inc(s_B, 16)
            # wp s2..s7 in three contiguous 256KB chunks (blocks 8.. of 384
            # cols; 1024 cols each = 2.67 blocks -> use flat col addressing)
            fl = ld_d.ap().rearrange("p g c -> p (g c)")
            fs = ld_sb.ap().rearrange("p g c -> p (g c)")
            sync.dma_start(fs[:, 3072:4096], fl[:, 3072:4096]).then_inc(s_wp23, 16)
            sync.dma_start(fs[:, 4096:5120], fl[:, 4096:5120]).then_inc(s_wp45, 16)
            sync.dma_start(fs[:, 5120:6144], fl[:, 5120:6144]).then_inc(s_wp67, 16)
            for j, s in enumerate((4, 5, 6)):
                sync.wait_ge(s_dve_sync, j + 1)
                sync.dma_start(yv[:, s, :], out_sb[:, s, :]).then_inc(s_st_sync, 16)
            sync.wait_ge(s_dve_sync, 4)
            sync.dma_start(yv[:, 7, 256:384], out_sb[:, 7, 256:384]).then_inc(
                s_st_sync, 16
            )
            # No final completion wait: the framework epilogue's DRAIN retires
            # in-flight DMAs, and its semaphore sweep re-zeroes every sem.

        @block.scalar
        def _(scalar):
            scalar.dma_start(cv_sb[:], cv_d.ap()).then_inc(s_cv, 16)
            for j, s in enumerate((0, 1, 2, 3)):
                scalar.wait_ge(s_dve_act, j + 1)
                scalar.dma_start(yv[:, s, :], out_sb[:, s, :]).then_inc(s_st_act, 16)
            scalar.wait_ge(s_dve_act, 5)
            scalar.dma_start(yv[:, 7, 0:256], out_sb[:, 7, 0:256]).then_inc(
                s_st_act, 16
            )
            # Final 64KB is split 2x32KB across both rings so the last
            # transfer after the last matmul is as short as possible.
            scalar.wait_ge(s_dve_sync, 4)
            scalar.dma_start(yv[:, 7, 384:512], out_sb[:, 7, 384:512]).then_inc(
                s_st_act, 16
            )

        @block.tensor
        def _(tensor):
            # Warm the PE HAM clock gate while loads are in flight. scr_sb is
            # never written on HW (garbage is fine - the warmup PSUM region is
            # overwritten with start=True before any read); the sim variant
            # memsets it because CoreSim rejects uninit reads.
            if not with_clears:
                tensor.wait_ge(s_ws, 1)
            for _ in range(N_WARM):
                nc.tensor.matmul(
                    ps[:, 7, 0:128],
                    scr_sb[:],
                    scr_sb[:],
                    start=True,
                    stop=True,
                )
            fs = ld_sb.ap().rearrange("p g c -> p (g c)")

            def wpk(s, k):
                # packed column of the 128-wide wp chunk (s, k)
                if s == 0:
                    return k * 768
                if s == 1:
                    return k * 768 + 640
                return 3072 + (s - 2) * 512 + k * 128

            XT = 128  # xt starts at col k*768+128; n1 half at +256

            # Group 0: two N=256 half-groups (chunk A has n0, chunk B n1).
            tensor.wait_ge(s_A, 16)
            for u in range(2):
                if u == 1:
                    tensor.wait_ge(s_B, 16)
                for k in range(KT):
                    inst = nc.tensor.matmul(
                        ps[:, 0, u * NQ:(u + 1) * NQ],
                        fs[:, wpk(0, k):wpk(0, k) + 128],
                        fs[:, k * 768 + XT + u * NQ:k * 768 + XT + (u + 1) * NQ],
                        start=(k == 0),
                        stop=(k == KT - 1),
                    )
                    if k == KT - 1:
                        inst.then_inc(s_mm, 1)
            # Groups 1-6: full N=512 matmuls (xt halves are adjacent).
            for s in range(1, ST - 1):
                if s == 2:
                    tensor.wait_ge(s_wp23, 16)
                elif s == 4:
                    tensor.wait_ge(s_wp45, 16)
                elif s == 6:
                    tensor.wait_ge(s_wp67, 16)
                for k in range(KT):
                    inst = nc.tensor.matmul(
                        ps[:, s, :],
                        fs[:, wpk(s, k):wpk(s, k) + 128],
                        fs[:, k * 768 + XT:k * 768 + XT + 512],
                        start=(k == 0),
                        stop=(k == KT - 1),
                    )
                    if k == KT - 1:
                        inst.then_inc(s_mm, 1)
            # Group 7 runs as two N=256 half-groups with the n0 half in bank
            # 0 (long closed), so its DVE add + store issue hide under the
            # last four matmuls and only the n1 half sits on the tail.
            # Bank 0 may only be overwritten once DVE's group-0 adds have
            # read it (s_dve_act reaches 1 right after; ~5us of slack).
            tensor.wait_ge(s_dve_act, 1)
            for u in range(2):
                for k in range(KT):
                    inst = nc.tensor.matmul(
                        ps[:, 7 * u, u * NQ:(u + 1) * NQ],
                        fs[:, wpk(7, k):wpk(7, k) + 128],
                        fs[:, k * 768 + XT + u * NQ:k * 768 + XT + (u + 1) * NQ],
                        start=(k == 0),
                        stop=(k == KT - 1),
                    )
                    if k == KT - 1:
                        inst.then_inc(s_mm, 1)

        @block.vector
        def _(vector):
            if not with_clears:
                vector.memset(scr_sb[:], 0).then_inc(s_ws, 1)
            vector.wait_ge(s_cv, 16)  # cv loaded
            # s_mm counts: group 0 incs twice (half-groups), groups 1-6 once
            # (cumulative s+2), group 7's halves inc 9 and 10. Each group is
            # added in two 256-wide halves (DVE pace 2x392ns < 864ns PE
            # group pace -> no tail backlog); inc map:
            #   s0-s3 h1 -> s_dve_act 1..4 (scalar-ring stores)
            #   s4-s6 h1 -> s_dve_sync 1..3 (sync-ring stores)
            #   s7 h0 -> s_dve_act 5, s7 h1 -> s_dve_sync 4 (split tail)
            for s in range(ST - 1):
                vector.wait_ge(s_mm, 2 if s == 0 else s + 2)
                nc.vector.tensor_scalar_add(
                    out_sb[:, s, :],
                    ps[:, s, :],
                    cv_sb[:, s:s + 1],
                ).then_inc(s_dve_act if s <= 3 else s_dve_sync, 1)
            for u in range(2):
                vector.wait_ge(s_mm, 9 + u)
                nc.vector.tensor_scalar_add(
                    out_sb[:, 7, u * NQ:(u + 1) * NQ],
                    ps[:, 7 * u, u * NQ:(u + 1) * NQ],
                    cv_sb[:, 7:8],
                ).then_inc(s_dve_act if u == 0 else s_dve_sync, 1)

    return nc


def _get_program():
    nc = _cache.get("nc")
    if nc is None:
        nc = _build_program()
        _cache["nc"] = nc
    return nc


def _prep_in_maps(x, idx, fbt, opt):
    bf = ml_dtypes.bfloat16
    in_maps = []
    for b in range(B):
        w = opt[idx[b]].reshape(F, D, O)                     # [F,D,O] f32
        wpack = w.transpose(1, 0, 2).reshape(KT, 128, ST, 128)  # [k,p,s,c]
        wp_host = np.ascontiguousarray(
            wpack.transpose(1, 2, 0, 3)                      # [p, s, k, c]
        ).astype(bf)                                         # [128, ST, KT, 128]
        bias = fbt[idx[b]]                                   # [F,D]
        cvec = np.einsum("fd,fdo->fo", bias, w).reshape(FO).astype(np.float32)
        cv = np.ascontiguousarray(cvec.reshape(ST, 128).T)   # [128, ST]
        for h in range(2):
            xtT = x[b, h * NH:(h + 1) * NH, :].T             # [D, NH]
            xt_host = (
                xtT.reshape(KT, 128, NH).transpose(1, 0, 2)
            ).astype(bf)                                     # [128, KT, NH=512]
            ld = np.empty((128, 6144), dtype=bf)
            for k in range(KT):
                base = k * 768
                ld[:, base:base + 128] = wp_host[:, 0, k]
                ld[:, base + 128:base + 640] = xt_host[:, k]
                ld[:, base + 640:base + 768] = wp_host[:, 1, k]
            ld[:, 3072:6144] = wp_host[:, 2:].reshape(128, 6 * 512)
            in_maps.append({"ld": ld.reshape(128, 16, 384), "cv": cv})
    return in_maps


def _assemble(results):
    out = np.empty((B, N, F, O), dtype=np.float32)
    for c in range(8):
        b, h = divmod(c, 2)
        y = np.asarray(results[c]["y"]).astype(np.float32)   # [FO, NH] bf16
        out[b, h * NH:(h + 1) * NH] = y.reshape(F, O, NH).transpose(2, 0, 1)
    return out


def _run(x, idx, feature_bias_table, out_projection_table, **run_kwargs):
    from concourse.bass_utils import run_bass_kernel_spmd

    x = np.asarray(x, dtype=np.float32)
    idx = np.asarray(idx).astype(np.int64)
    fbt = np.asarray(feature_bias_table, dtype=np.float32)
    opt = np.asarray(out_projection_table, dtype=np.float32)

    nc = _get_program()
    in_maps = _prep_in_maps(x, idx, fbt, opt)
    res = run_bass_kernel_spmd(nc, in_maps, core_ids=list(range(8)), **run_kwargs)
    return _assemble(res.results), res


def kernel(x, idx, feature_bias_table, out_projection_table):
    out, _ = _run(x, idx, feature_bias_table, out_projection_table)
    return out


# revision 21
# speedup vs baseline: 1.1434x; 1.1434x over previous
"""Trainium2 Bass kernel for nn_ModalDecoder (embedding_lookup).

Reference computation:
    w  = out_projection_table[idx].reshape(B, F, D, O)      # [B,F,D,O]
    b  = feature_bias_table[idx]                            # [B,F,D]
    xb = x[:, :, None, :] + b[:, None, :, :]                # [B,N,F,D]
    out = einsum('bnfd,bfdo->bnfo', xb, w)                  # [B,N,F,O]

Factorization (avoids the 128MB [B,N,F,D] intermediate):
    out[b, n, f, :] = x[b, n, :] @ W[b, f] + (bias[b, f] @ W[b, f])
The bias term is a per-(b,f) length-O vector, broadcast over n; it is
precomputed on host (tiny) and added on-device per PSUM tile via
tensor_scalar_add (per-partition scalar), which doubles as the mandatory
PSUM->SBUF copy (DMA cannot read PSUM).

Sharding: 8 cores = 4 values of b x 2 halves of N. Per core:
    y[fo, n] = Wpack[d, fo].T @ xT[d, n] + cvec[fo]
with Wpack = [D, F*O] (host-gathered tables packed side by side), xT the
transposed x half, both bf16 (PSUM accumulates fp32). y is [F*O, NH] bf16
(host upcasts; bf16 output keeps rel err ~3e-3 vs the 2e-2 gate and halves
store traffic to 1MB/core).

Schedule (from perfetto analysis; ~7us of framework preamble and ~1.7us of
epilogue bracket everything):
  * ALL bulk load data lives in ONE packed DRAM tensor on the sync HWDGE
    ring. Descriptor generation costs ~0.65us per dma_start and the
    generation backend is shared between the two HWDGE rings (splitting
    loads across rings parallelizes nothing - measured repeatedly), so
    chunk count and boundaries matter, not tensor boundaries. The first
    3072 columns are four per-k regions [wp_s0_k | xt_k_n0 | xt_k_n1 |
    wp_s1_k] so that chunk A (one 4x768B-run strided DMA, 384KB) delivers
    wp0 + the xt n0-half and gates group 0, chunk B (same shape) delivers
    the xt n1-half + wp1 and gates both s0u1 and s1, while the N=512 rhs
    (both xt halves) stays contiguous per k. wp for s2..s7 follows in
    three 256KB chunks that pace groups 2-7 just ahead of the PE. 5 gens
    instead of 7 pulls every gate ~0.6us earlier.
  * cv rides the scalar ring alone (it gates every DVE add; behind the wp
    bytes it arrives microseconds late and stalls the whole add pipeline).
  * Group 0 runs as two N=256 half-groups (start on 384KB landed);
    groups 1-6 run as N=512 (warm pace 216ns vs 2x131ns - LDWEIGHTS is
    not fully hidden at N=256).
  * Group 7 runs as two N=256 half-groups with the n0 half accumulated in
    bank 0 (closed and drained ~5us earlier; a PE wait on s_dve_act guards
    the reuse), so its DVE add + store issue overlap the last 4 matmuls
    and only the 64KB n1 half sits on the post-matmul tail.
  * The PE is pre-warmed with N=128 dummy matmuls (cold issue ~107ns each)
    sized to run FLUSH into the first gate (~4.1us), so the HAM clock gate
    flips to 2.4GHz before the first real group and every real matmul runs
    warm. Even a ~0.5us warmup->gate idle gap can leave HAM cold for
    microseconds (free-running window phase); undersizing by 2us once
    cost ~4us of half-rate matmuls.
  * DVE does one 512-wide add per group (751ns+overhead < 864ns PE group
    pace; two 256-wide halves per group measured ~1.0us/group and built a
    ~1.8us tail backlog). Group 7 keeps 256-wide half-adds for the split
    tail.
  * Stores alternate rings: scalar takes s0-s3 + s7n0, sync takes s4-s6 +
    s7n1 after its loads.
  * No end-of-kernel waits or sem clears: the NEFF epilogue drains DMAs
    and re-zeroes every semaphore, overlapping the last store transfer.

Per-core HBM traffic: 1.5MB packed loads + 1MB out (memory-bound).
"""

import numpy as np
import ml_dtypes

B, N, D, O, F, V = 4, 1024, 512, 64, 16, 64
NH = N // 2            # 512 rows of x per core
FO = F * O             # 1024 packed output columns
KT = D // 128          # 4 contraction chunks
ST = FO // 128         # 8 output-partition chunks
NQ = NH // 2           # 256 (n-half within a core, group-0/7 split)
N_WARM = 38            # short N=128 PE warmup matmuls during load phase

_cache: dict = {}


def _build_program(with_clears=True):
    # with_clears=True is the real (HW) program. The False variant is for
    # CoreSim validation: it enables the race detector and memsets the
    # warmup scratch (CoreSim rejects reads of uninitialized SBUF; on HW
    # the warmup matmul inputs are garbage by design and never observed).
    import concourse.bass as bass
    import concourse.mybir as mybir

    bf16 = mybir.dt.bfloat16
    f32 = mybir.dt.float32

    nc = bass.Bass(
        "TRN2",
        target_bir_lowering=False,
        debug=False,
        num_devices=8,
        detect_race_conditions=not with_clears,
    )

    # Packed load tensor, 6144 columns. First 3072: four per-k regions of
    # 768 cols [wp_s0_k (128) | xt_k_n0 (256) | xt_k_n1 (256) | wp_s1_k
    # (128)], so ONE 4-run strided DMA covers wp0+xt_n0 (chunk A, gates
    # group 0) and another covers xt_n1+wp1 (chunk B, gates s0u1 AND s1),
    # while the N=512 rhs (xt both halves) stays contiguous per k.
    # Last 3072: wp for s=2..7 as s*512+k*128+c.
    ld_d = nc.dram_tensor("ld", [128, 16, 384], bf16, kind="ExternalInput")
    cv_d = nc.dram_tensor("cv", [128, ST], f32, kind="ExternalInput")
    y_d = nc.dram_tensor("y", [FO, NH], bf16, kind="ExternalOutput")

    yv = y_d.ap().rearrange("(g p) n -> p g n", p=128)  # [128, ST, NH]

    with (
        nc.sbuf_tensor("ld_sb", [128, 16, 384], bf16) as ld_sb,
        nc.sbuf_tensor("cv_sb", [128, ST], f32) as cv_sb,
        nc.sbuf_tensor("out_sb", [128, ST, NH], bf16) as out_sb,
        nc.sbuf_tensor("scr_sb", [128, 128], bf16) as scr_sb,
        nc.psum_tensor([128, ST, NH], f32) as ps,
        nc.semaphore("s_A") as s_A,
        nc.semaphore("s_B") as s_B,
        nc.semaphore("s_wp23") as s_wp23,
        nc.semaphore("s_wp45") as s_wp45,
        nc.semaphore("s_wp67") as s_wp67,
        nc.semaphore("s_cv") as s_cv,
        nc.semaphore("s_mm") as s_mm,
        nc.semaphore("s_dve_sync") as s_dve_sync,
        nc.semaphore("s_dve_act") as s_dve_act,
        nc.semaphore("s_st_sync") as s_st_sync,   # store completion (never waited)
        nc.semaphore("s_st_act") as s_st_act,
        nc.semaphore("s_ws") as s_ws,
        nc.Block() as block,
    ):

        @block.sync
        def _(sync):
            lv = ld_sb.ap().rearrange("p (g h) c -> p g h c", h=2)
            dv = ld_d.ap().rearrange("p (g h) c -> p g h c", h=2)
            # chunk A: [wp_s0_k | xt_k_n0] for all k (4x768B runs, 384KB)
            sync.dma_start(lv[:, 0:4, 0, :], dv[:, 0:4, 0, :]).then_inc(s_A, 16)
            # chunk B: [xt_k_n1 | wp_s1_k] for all k (gates s0u1 AND s1)
            sync.dma_start(lv[:, 0:4, 1, :], dv[:, 0:4, 1, :]).then_inc(s_B, 16)
            # wp s2..s7 in three contiguous 256KB chunks (blocks 8.. of 384
            # cols; 1024 cols each = 2.67 blocks -> use flat col addressing)
            fl = ld_d.ap().rearrange("p g c -> p (g c)")
            fs = ld_sb.ap().rearrange("p g c -> p (g c)")
            sync.dma_start(fs[:, 3072:4096], fl[:, 3072:4096]).then_inc(s_wp23, 16)
            sync.dma_start(fs[:, 4096:5120], fl[:, 4096:5120]).then_inc(s_wp45, 16)
            sync.dma_start(fs[:, 5120:6144], fl[:, 5120:6144]).then_inc(s_wp67, 16)
            for j, s in enumerate((4, 5, 6)):
                sync.wait_ge(s_dve_sync, j + 1)
                sync.dma_start(yv[:, s, :], out_sb[:, s, :]).then_inc(s_st_sync, 16)
            sync.wait_ge(s_dve_sync, 4)
            sync.dma_start(yv[:, 7, 256:512], out_sb[:, 7, 256:512]).then_inc(
                s_st_sync, 16
            )
            # No final completion wait: the framework epilogue's DRAIN retires
            # in-flight DMAs, and its semaphore sweep re-zeroes every sem.

        @block.scalar
        def _(scalar):
            scalar.dma_start(cv_sb[:], cv_d.ap()).then_inc(s_cv, 16)
            for j, s in enumerate((0, 1, 2, 3)):
                scalar.wait_ge(s_dve_act, j + 1)
                scalar.dma_start(yv[:, s, :], out_sb[:, s, :]).then_inc(s_st_act, 16)
            scalar.wait_ge(s_dve_act, 5)
            scalar.dma_start(yv[:, 7, 0:256], out_sb[:, 7, 0:256]).then_inc(
                s_st_act, 16
            )

        @block.tensor
        def _(tensor):
            # Warm the PE HAM clock gate while loads are in flight. scr_sb is
            # never written on HW (garbage is fine - the warmup PSUM region is
            # overwritten with start=True before any read); the sim variant
            # memsets it because CoreSim rejects uninit reads.
            if not with_clears:
                tensor.wait_ge(s_ws, 1)
            for _ in range(N_WARM):
                nc.tensor.matmul(
                    ps[:, 7, 0:128],
                    scr_sb[:],
                    scr_sb[:],
                    start=True,
                    stop=True,
                )
            fs = ld_sb.ap().rearrange("p g c -> p (g c)")

            def wpk(s, k):
                # packed column of the 128-wide wp chunk (s, k)
                if s == 0:
                    return k * 768
                if s == 1:
                    return k * 768 + 640
                return 3072 + (s - 2) * 512 + k * 128

            XT = 128  # xt starts at col k*768+128; n1 half at +256

            # Group 0: two N=256 half-groups (chunk A has n0, chunk B n1).
            tensor.wait_ge(s_A, 16)
            for u in range(2):
                if u == 1:
                    tensor.wait_ge(s_B, 16)
                for k in range(KT):
                    inst = nc.tensor.matmul(
                        ps[:, 0, u * NQ:(u + 1) * NQ],
                        fs[:, wpk(0, k):wpk(0, k) + 128],
                        fs[:, k * 768 + XT + u * NQ:k * 768 + XT + (u + 1) * NQ],
                        start=(k == 0),
                        stop=(k == KT - 1),
                    )
                    if k == KT - 1:
                        inst.then_inc(s_mm, 1)
            # Groups 1-6: full N=512 matmuls (xt halves are adjacent).
            for s in range(1, ST - 1):
                if s == 2:
                    tensor.wait_ge(s_wp23, 16)
                elif s == 4:
                    tensor.wait_ge(s_wp45, 16)
                elif s == 6:
                    tensor.wait_ge(s_wp67, 16)
                for k in range(KT):
                    inst = nc.tensor.matmul(
                        ps[:, s, :],
                        fs[:, wpk(s, k):wpk(s, k) + 128],
                        fs[:, k * 768 + XT:k * 768 + XT + 512],
                        start=(k == 0),
                        stop=(k == KT - 1),
                    )
                    if k == KT - 1:
                        inst.then_inc(s_mm, 1)
            # Group 7 runs as two N=256 half-groups with the n0 half in bank
            # 0 (long closed), so its DVE add + store issue hide under the
            # last four matmuls and only the n1 half sits on the tail.
            # Bank 0 may only be overwritten once DVE's group-0 adds have
            # read it (s_dve_act reaches 1 right after; ~5us of slack).
            tensor.wait_ge(s_dve_act, 1)
            for u in range(2):
                for k in range(KT):
                    inst = nc.tensor.matmul(
                        ps[:, 7 * u, u * NQ:(u + 1) * NQ],
                        fs[:, wpk(7, k):wpk(7, k) + 128],
                        fs[:, k * 768 + XT + u * NQ:k * 768 + XT + (u + 1) * NQ],
                        start=(k == 0),
                        stop=(k == KT - 1),
                    )
                    if k == KT - 1:
                        inst.then_inc(s_mm, 1)

        @block.vector
        def _(vector):
            if not with_clears:
                vector.memset(scr_sb[:], 0).then_inc(s_ws, 1)
            vector.wait_ge(s_cv, 16)  # cv loaded
            # s_mm counts: group 0 incs twice (half-groups), groups 1-6 once
            # (cumulative s+2), group 7's halves inc 9 and 10. Each group is
            # added in two 256-wide halves (DVE pace 2x392ns < 864ns PE
            # group pace -> no tail backlog); inc map:
            #   s0-s3 h1 -> s_dve_act 1..4 (scalar-ring stores)
            #   s4-s6 h1 -> s_dve_sync 1..3 (sync-ring stores)
            #   s7 h0 -> s_dve_act 5, s7 h1 -> s_dve_sync 4 (split tail)
            for s in range(ST - 1):
                vector.wait_ge(s_mm, 2 if s == 0 else s + 2)
                nc.vector.tensor_scalar_add(
                    out_sb[:, s, :],
                    ps[:, s, :],
                    cv_sb[:, s:s + 1],
                ).then_inc(s_dve_act if s <= 3 else s_dve_sync, 1)
            for u in range(2):
                vector.wait_ge(s_mm, 9 + u)
                nc.vector.tensor_scalar_add(
                    out_sb[:, 7, u * NQ:(u + 1) * NQ],
                    ps[:, 7 * u, u * NQ:(u + 1) * NQ],
                    cv_sb[:, 7:8],
                ).then_inc(s_dve_act if u == 0 else s_dve_sync, 1)

    return nc


def _get_program():
    nc = _cache.get("nc")
    if nc is None:
        nc = _build_program()
        _cache["nc"] = nc
    return nc


def _prep_in_maps(x, idx, fbt, opt):
    bf = ml_dtypes.bfloat16
    in_maps = []
    for b in range(B):
        w = opt[idx[b]].reshape(F, D, O)                     # [F,D,O] f32
        wpack = w.transpose(1, 0, 2).reshape(KT, 128, ST, 128)  # [k,p,s,c]
        wp_host = np.ascontiguousarray(
            wpack.transpose(1, 2, 0, 3)                      # [p, s, k, c]
        ).astype(bf)                                         # [128, ST, KT, 128]
        bias = fbt[idx[b]]                                   # [F,D]
        cvec = np.einsum("fd,fdo->fo", bias, w).reshape(FO).astype(np.float32)
        cv = np.ascontiguousarray(cvec.reshape(ST, 128).T)   # [128, ST]
        for h in range(2):
            xtT = x[b, h * NH:(h + 1) * NH, :].T             # [D, NH]
            xt_host = (
                xtT.reshape(KT, 128, NH).transpose(1, 0, 2)
            ).astype(bf)                                     # [128, KT, NH=512]
            ld = np.empty((128, 6144), dtype=bf)
            for k in range(KT):
                base = k * 768
                ld[:, base:base + 128] = wp_host[:, 0, k]
                ld[:, base + 128:base + 640] = xt_host[:, k]
                ld[:, base + 640:base + 768] = wp_host[:, 1, k]
            ld[:, 3072:6144] = wp_host[:, 2:].reshape(128, 6 * 512)
            in_maps.append({"ld": ld.reshape(128, 16, 384), "cv": cv})
    return in_maps


def _assemble(results):
    out = np.empty((B, N, F, O), dtype=np.float32)
    for c in range(8):
        b, h = divmod(c, 2)
        y = np.asarray(results[c]["y"]).astype(np.float32)   # [FO, NH] bf16
        out[b, h * NH:(h + 1) * NH] = y.reshape(F, O, NH).transpose(2, 0, 1)
    return out


def _run(x, idx, feature_bias_table, out_projection_table, **run_kwargs):
    from concourse.bass_utils import run_bass_kernel_spmd

    x = np.asarray(x, dtype=np.float32)
    idx = np.asarray(idx).astype(np.int64)
    fbt = np.asarray(feature_bias_table, dtype=np.float32)
    opt = np.asarray(out_projection_table, dtype=np.float32)

    nc = _get_program()
    in_maps = _prep_in_maps(x, idx, fbt, opt)
    res = run_bass_kernel_spmd(nc, in_maps, core_ids=list(range(8)), **run_kwargs)
    return _assemble(res.results), res


def kernel(x, idx, feature_bias_table, out_projection_table):
    out, _ = _run(x, idx, feature_bias_table, out_projection_table)
    return out
